# revision 5
# baseline (speedup 1.0000x reference)
"""GridLayer.get_nh neighborhood gather — TRN2 Bass kernel, 8-core SPMD.

out[b, n, k, 0, :] = x[b, adjc[local_indices[b, n], k] - offset[b], 0, :]
mask[b, n, k, 0]   = adjc_mask_invalid[local_indices[b, n], k]

Sharding: each of the 8 cores owns batch b = core//2 and half of the n
range (32768 rows -> 229376 gathered rows of 512B each).

Device kernel: InstDMAGatherAnt (gpsimd dma_gather, mlp ucode library).
Each instruction gathers 3584 rows by int16 index from a 16MB half of
the x table (int16 range forces a lo/hi table split; gather-rows are
partitioned by half on the host, padded with dummy index 0 to a fixed
33+33 instruction grid). Indices are wrapped [16, n/16] and replicated
across the eight 16-partition groups (the Q7 ucode streams them from a
queue-dependent partition group). Gathered tiles bounce through SBUF
and are written back as one strided DMA per instruction; the host
un-permutes rows during the (mandatory) per-core assembly copy.
"""

import numpy as np

import concourse.tile as tile
from concourse import bacc, bass, library_config, mybir
from concourse.bass import AP
from concourse.bass_utils import run_bass_kernel_spmd

B, N, NV, E, NH = 4, 65536, 1, 128, 7
GLOBAL_LEVEL = 0

NCORES = 8
SPB = NCORES // B          # core-shards per batch element = 2
NS = N // SPB              # output rows per core = 32768
ROWS = NS * NH             # gathered rows per core = 229376
NI = 3584                  # gathered rows per dma_gather instruction
G = NI // 128              # chunks per partition = 28
S = NI // 16               # wrapped idx columns per instruction = 224
TL = 33                    # lo-table instructions (33*3584 = 118272 >= nlo)
TH = 33                    # hi-table instructions
TT = TL + TH
HALF = N // 2              # 32768 rows per table half
GBUFS = 4                  # pipeline depth of gather/writeback tiles

_CACHE: dict = {}
LAST_RESULTS = None        # test harness introspection


def _build() -> bacc.Bacc:
    nc = bacc.Bacc("TRN2", target_bir_lowering=False, debug=False)
    xb = nc.dram_tensor("xb", [N, E], mybir.dt.float32, kind="ExternalInput").ap()
    idxt = nc.dram_tensor("idxt", [128, TT * S], mybir.dt.int16, kind="ExternalInput").ap()
    y = nc.dram_tensor("y", [TT * NI, E], mybir.dt.float32, kind="ExternalOutput").ap()

    with tile.TileContext(nc) as tc:
        with tc.tile_pool(name="cst", bufs=1) as cst, tc.tile_pool(name="gp", bufs=GBUFS) as gp:
            nc.gpsimd.load_library(library_config.mlp)
            idx_sb = cst.tile([128, TT * S], mybir.dt.int16)
            nc.sync.dma_start(out=idx_sb[:], in_=idxt[:])
            for t in range(TT):
                src = xb[:HALF, :] if t < TL else xb[HALF:, :]
                g = gp.tile([128, G, E], mybir.dt.float32, name="g")
                nc.gpsimd.dma_gather(
                    out_ap=g[:],
                    in_ap=src,
                    idxs_ap=idx_sb[:, t * S:(t + 1) * S],
                    num_idxs=NI,
                    num_idxs_reg=NI,
                    elem_size=E,
                    single_packet=False,
                )
                dram = AP(y.tensor, t * NI * E, [[E, 128], [128 * E, G], [1, E]])
                nc.sync.dma_start(out=dram, in_=g[:])
    nc.compile()
    return nc


def _get_nc() -> bacc.Bacc:
    if "nc" not in _CACHE:
        _CACHE["nc"] = _build()
    return _CACHE["nc"]


def _prep_core(flat_idx: np.ndarray):
    """flat_idx [ROWS] int64/int32 in [0, N) -> (idxt [128, TT*S] int16,
    ypos [ROWS] int32: output row r lives at y[ypos[r]])."""
    is_hi = flat_idx >= HALF
    lo_pos = np.flatnonzero(~is_hi)
    hi_pos = np.flatnonzero(is_hi)
    nlo, nhi = len(lo_pos), len(hi_pos)
    assert nlo <= TL * NI and nhi <= TH * NI, (nlo, nhi)

    vals = np.zeros(TT * NI, dtype=np.int16)
    vals[:nlo] = flat_idx[lo_pos]
    vals[TL * NI:TL * NI + nhi] = flat_idx[hi_pos] - HALF

    # out[p, g] of instr t <- vals[t*NI + g*128 + p]; wrapped [16, S] rep x8
    w16 = vals.reshape(TT, S, 16).transpose(0, 2, 1)        # [TT, 16, S]
    idxt = np.tile(w16, (1, 8, 1)).transpose(1, 0, 2).reshape(128, TT * S)

    ypos = np.empty(ROWS, dtype=np.int64)
    ypos[lo_pos] = np.arange(nlo)
    ypos[hi_pos] = TL * NI + np.arange(nhi)
    return np.ascontiguousarray(idxt), ypos


def kernel(x, adjc, adjc_mask_invalid, local_indices, batch_sample_indices,
           sampled_level, _trace=False):
    global LAST_RESULTS
    x = np.asarray(x)
    adjc = np.asarray(adjc)
    adjc_mask_invalid = np.asarray(adjc_mask_invalid)
    local_indices = np.asarray(local_indices)
    batch_sample_indices = np.asarray(batch_sample_indices)

    # Host-side index math (tiny vs the 939MB gather itself).
    offset = batch_sample_indices.astype(np.int64) * (
        4 ** (int(sampled_level) - GLOBAL_LEVEL)
    )
    indices_nh = adjc[local_indices]                      # [B, N, NH]
    idx = (indices_nh.astype(np.int64) - offset[:, None, None])

    in_maps, yposs = [], []
    for c in range(NCORES):
        b, s = divmod(c, SPB)
        flat = idx[b, s * NS:(s + 1) * NS].reshape(ROWS)
        idxt, ypos = _prep_core(flat)
        xb = np.ascontiguousarray(x[b].reshape(N, E))
        in_maps.append({"xb": xb, "idxt": idxt})
        yposs.append(ypos)

    nc = _get_nc()
    res = run_bass_kernel_spmd(
        nc, in_maps, core_ids=list(range(NCORES)), trace=_trace
    )
    LAST_RESULTS = res

    x_nh = np.empty((B, N, NH, NV, E), dtype=np.float32)
    for c in range(NCORES):
        b, s = divmod(c, SPB)
        dst = x_nh[b, s * NS:(s + 1) * NS].reshape(ROWS, E)
        np.take(res.results[c]["y"], yposs[c], axis=0, out=dst)

    nh_mask = adjc_mask_invalid[local_indices]            # [B, N, NH]
    mask = np.broadcast_to(nh_mask[..., None], (B, N, NH, NV)).copy()
    return x_nh, mask


# revision 7
# speedup vs baseline: 1.6408x; 1.6408x over previous
"""GridLayer.get_nh neighborhood gather — TRN2 Bass kernel, 8-core SPMD.

out[b, n, k, 0, :] = x[b, adjc[local_indices[b, n], k] - offset[b], 0, :]
mask[b, n, k, 0]   = adjc_mask_invalid[local_indices[b, n], k]

Sharding: each of the 8 cores owns batch b = core//2 and half of the n
range (32768 rows -> 229376 gathered rows of 512B each).

Device kernel: InstDMAGatherAnt (gpsimd dma_gather, mlp ucode library).
Each instruction gathers 3584 rows by int16 index from a 16MB half of
the x table (int16 range forces a lo/hi table split; gather-rows are
partitioned by half on the host, padded with dummy index 0 to a fixed
33+33 instruction grid). Indices are wrapped [16, n/16] and replicated
across the eight 16-partition groups (the Q7 ucode streams them from a
queue-dependent partition group). Gathered tiles bounce through SBUF
and are written back as one strided DMA per instruction; the host
un-permutes rows during the (mandatory) per-core assembly copy.
"""

import numpy as np

import concourse.tile as tile
from concourse import bacc, bass, library_config, mybir
from concourse.bass import AP
from concourse.bass_utils import run_bass_kernel_spmd

B, N, NV, E, NH = 4, 65536, 1, 128, 7
GLOBAL_LEVEL = 0

NCORES = 8
SPB = NCORES // B          # core-shards per batch element = 2
NS = N // SPB              # output rows per core = 32768
ROWS = NS * NH             # gathered rows per core = 229376
NI = 3584                  # gathered rows per dma_gather instruction
G = NI // 128              # chunks per partition = 28
S = NI // 16               # wrapped idx columns per instruction = 224
TL = 33                    # lo-table instructions (33*3584 = 118272 >= nlo)
TH = 33                    # hi-table instructions
TT = TL + TH
HALF = N // 2              # 32768 rows per table half
GBUFS = 4                  # pipeline depth of gather/writeback tiles

_CACHE: dict = {}
LAST_RESULTS = None        # test harness introspection


def _build() -> bacc.Bacc:
    nc = bacc.Bacc("TRN2", target_bir_lowering=False, debug=False,
                   num_swdge_queues=4)
    xb = nc.dram_tensor("xb", [N, E], mybir.dt.float32, kind="ExternalInput").ap()
    idxt = nc.dram_tensor("idxt", [128, TT * S], mybir.dt.int16, kind="ExternalInput").ap()
    y = nc.dram_tensor("y", [TT * NI, E], mybir.dt.float32, kind="ExternalOutput").ap()

    with tile.TileContext(nc) as tc:
        with tc.tile_pool(name="cst", bufs=1) as cst, tc.tile_pool(name="gp", bufs=GBUFS) as gp:
            nc.gpsimd.load_library(library_config.mlp)
            idx_sb = cst.tile([128, TT * S], mybir.dt.int16)
            nc.sync.dma_start(out=idx_sb[:], in_=idxt[:])
            for t in range(TT):
                src = xb[:HALF, :] if t < TL else xb[HALF:, :]
                g = gp.tile([128, G, E], mybir.dt.float32, name="g")
                nc.gpsimd.dma_gather(
                    out_ap=g[:],
                    in_ap=src,
                    idxs_ap=idx_sb[:, t * S:(t + 1) * S],
                    num_idxs=NI,
                    num_idxs_reg=NI,
                    elem_size=E,
                    single_packet=False,
                    queue_num=t % 4,
                )
                dram = AP(y.tensor, t * NI * E, [[E, 128], [128 * E, G], [1, E]])
                nc.sync.dma_start(out=dram, in_=g[:])
    nc.compile()
    return nc


def _get_nc() -> bacc.Bacc:
    if "nc" not in _CACHE:
        _CACHE["nc"] = _build()
    return _CACHE["nc"]


def _prep_core(flat_idx: np.ndarray):
    """flat_idx [ROWS] int64/int32 in [0, N) -> (idxt [128, TT*S] int16,
    ypos [ROWS] int32: output row r lives at y[ypos[r]])."""
    is_hi = flat_idx >= HALF
    lo_pos = np.flatnonzero(~is_hi)
    hi_pos = np.flatnonzero(is_hi)
    nlo, nhi = len(lo_pos), len(hi_pos)
    assert nlo <= TL * NI and nhi <= TH * NI, (nlo, nhi)

    vals = np.zeros(TT * NI, dtype=np.int16)
    vals[:nlo] = flat_idx[lo_pos]
    vals[TL * NI:TL * NI + nhi] = flat_idx[hi_pos] - HALF

    # out[p, g] of instr t <- vals[t*NI + g*128 + p]; wrapped [16, S] rep x8
    w16 = vals.reshape(TT, S, 16).transpose(0, 2, 1)        # [TT, 16, S]
    idxt = np.tile(w16, (1, 8, 1)).transpose(1, 0, 2).reshape(128, TT * S)

    ypos = np.empty(ROWS, dtype=np.int64)
    ypos[lo_pos] = np.arange(nlo)
    ypos[hi_pos] = TL * NI + np.arange(nhi)
    return np.ascontiguousarray(idxt), ypos


def kernel(x, adjc, adjc_mask_invalid, local_indices, batch_sample_indices,
           sampled_level, _trace=False):
    global LAST_RESULTS
    x = np.asarray(x)
    adjc = np.asarray(adjc)
    adjc_mask_invalid = np.asarray(adjc_mask_invalid)
    local_indices = np.asarray(local_indices)
    batch_sample_indices = np.asarray(batch_sample_indices)

    # Host-side index math (tiny vs the 939MB gather itself).
    offset = batch_sample_indices.astype(np.int64) * (
        4 ** (int(sampled_level) - GLOBAL_LEVEL)
    )
    indices_nh = adjc[local_indices]                      # [B, N, NH]
    idx = (indices_nh.astype(np.int64) - offset[:, None, None])

    in_maps, yposs = [], []
    for c in range(NCORES):
        b, s = divmod(c, SPB)
        flat = idx[b, s * NS:(s + 1) * NS].reshape(ROWS)
        idxt, ypos = _prep_core(flat)
        xb = np.ascontiguousarray(x[b].reshape(N, E))
        in_maps.append({"xb": xb, "idxt": idxt})
        yposs.append(ypos)

    nc = _get_nc()
    res = run_bass_kernel_spmd(
        nc, in_maps, core_ids=list(range(NCORES)), trace=_trace
    )
    LAST_RESULTS = res

    x_nh = np.empty((B, N, NH, NV, E), dtype=np.float32)
    for c in range(NCORES):
        b, s = divmod(c, SPB)
        dst = x_nh[b, s * NS:(s + 1) * NS].reshape(ROWS, E)
        np.take(res.results[c]["y"], yposs[c], axis=0, out=dst)

    nh_mask = adjc_mask_invalid[local_indices]            # [B, N, NH]
    mask = np.broadcast_to(nh_mask[..., None], (B, N, NH, NV)).copy()
    return x_nh, mask


# revision 9
# speedup vs baseline: 2.1232x; 1.2940x over previous
"""GridLayer.get_nh neighborhood gather — TRN2 Bass kernel, 8-core SPMD.

out[b, n, k, 0, :] = x[b, adjc[local_indices[b, n], k] - offset[b], 0, :]
mask[b, n, k, 0]   = adjc_mask_invalid[local_indices[b, n], k]

Sharding: each of the 8 cores owns batch b = core//2 and half of the n
range (32768 rows -> 229376 gathered rows of 512B each).

Device kernel: InstDMAGatherAnt (gpsimd dma_gather, mlp ucode library).
Each instruction gathers 3584 rows by int16 index from a 16MB half of
the x table (int16 range forces a lo/hi table split; gather-rows are
partitioned by half on the host, padded with dummy index 0 to a fixed
33+33 instruction grid). Indices are wrapped [16, n/16] and replicated
across the eight 16-partition groups (the Q7 ucode streams them from a
queue-dependent partition group). Gathered tiles bounce through SBUF
and are written back as one strided DMA per instruction; the host
un-permutes rows during the (mandatory) per-core assembly copy.
"""

import numpy as np

import concourse.tile as tile
from concourse import bacc, bass, library_config, mybir
from concourse.bass import AP
from concourse.bass_utils import run_bass_kernel_spmd

B, N, NV, E, NH = 4, 65536, 1, 128, 7
GLOBAL_LEVEL = 0

NCORES = 8
SPB = NCORES // B          # core-shards per batch element = 2
NS = N // SPB              # output rows per core = 32768
ROWS = NS * NH             # gathered rows per core = 229376
NI = 3584                  # gathered rows per dma_gather instruction
G = NI // 128              # chunks per partition = 28
S = NI // 16               # wrapped idx columns per instruction = 224
TL = 33                    # lo-table instructions (33*3584 = 118272 >= nlo)
TH = 33                    # hi-table instructions
TT = TL + TH
HALF = N // 2              # 32768 rows per table half
GBUFS = 8                  # pipeline depth of gather/writeback tiles

_CACHE: dict = {}
LAST_RESULTS = None        # test harness introspection


def _build() -> bacc.Bacc:
    nc = bacc.Bacc("TRN2", target_bir_lowering=False, debug=False,
                   num_swdge_queues=4, dynamic_dma_scratch_size=2**15)
    xb = nc.dram_tensor("xb", [N, E], mybir.dt.float32, kind="ExternalInput").ap()
    idxt = nc.dram_tensor("idxt", [128, TT * S], mybir.dt.int16, kind="ExternalInput").ap()
    y = nc.dram_tensor("y", [TT * NI, E], mybir.dt.float32, kind="ExternalOutput").ap()

    with tile.TileContext(nc) as tc:
        with tc.tile_pool(name="cst", bufs=1) as cst, tc.tile_pool(name="gp", bufs=GBUFS) as gp:
            nc.gpsimd.load_library(library_config.mlp)
            idx_sb = cst.tile([128, TT * S], mybir.dt.int16)
            nc.sync.dma_start(out=idx_sb[:], in_=idxt[:])
            for t in range(TT):
                src = xb[:HALF, :] if t < TL else xb[HALF:, :]
                g = gp.tile([128, G, E], mybir.dt.float32, name="g")
                nc.gpsimd.dma_gather(
                    out_ap=g[:],
                    in_ap=src,
                    idxs_ap=idx_sb[:, t * S:(t + 1) * S],
                    num_idxs=NI,
                    num_idxs_reg=NI,
                    elem_size=E,
                    single_packet=False,
                    queue_num=t % 4,
                )
                dram = AP(y.tensor, t * NI * E, [[E, 128], [128 * E, G], [1, E]])
                nc.sync.dma_start(out=dram, in_=g[:])
    nc.compile()
    return nc


def _get_nc() -> bacc.Bacc:
    if "nc" not in _CACHE:
        _CACHE["nc"] = _build()
    return _CACHE["nc"]


def _prep_core(flat_idx: np.ndarray):
    """flat_idx [ROWS] int64/int32 in [0, N) -> (idxt [128, TT*S] int16,
    ypos [ROWS] int32: output row r lives at y[ypos[r]])."""
    is_hi = flat_idx >= HALF
    lo_pos = np.flatnonzero(~is_hi)
    hi_pos = np.flatnonzero(is_hi)
    nlo, nhi = len(lo_pos), len(hi_pos)
    assert nlo <= TL * NI and nhi <= TH * NI, (nlo, nhi)

    vals = np.zeros(TT * NI, dtype=np.int16)
    vals[:nlo] = flat_idx[lo_pos]
    vals[TL * NI:TL * NI + nhi] = flat_idx[hi_pos] - HALF

    # out[p, g] of instr t <- vals[t*NI + g*128 + p]; wrapped [16, S] rep x8
    w16 = vals.reshape(TT, S, 16).transpose(0, 2, 1)        # [TT, 16, S]
    idxt = np.tile(w16, (1, 8, 1)).transpose(1, 0, 2).reshape(128, TT * S)

    ypos = np.empty(ROWS, dtype=np.int64)
    ypos[lo_pos] = np.arange(nlo)
    ypos[hi_pos] = TL * NI + np.arange(nhi)
    return np.ascontiguousarray(idxt), ypos


def kernel(x, adjc, adjc_mask_invalid, local_indices, batch_sample_indices,
           sampled_level, _trace=False):
    global LAST_RESULTS
    x = np.asarray(x)
    adjc = np.asarray(adjc)
    adjc_mask_invalid = np.asarray(adjc_mask_invalid)
    local_indices = np.asarray(local_indices)
    batch_sample_indices = np.asarray(batch_sample_indices)

    # Host-side index math (tiny vs the 939MB gather itself).
    offset = batch_sample_indices.astype(np.int64) * (
        4 ** (int(sampled_level) - GLOBAL_LEVEL)
    )
    indices_nh = adjc[local_indices]                      # [B, N, NH]
    idx = (indices_nh.astype(np.int64) - offset[:, None, None])

    in_maps, yposs = [], []
    for c in range(NCORES):
        b, s = divmod(c, SPB)
        flat = idx[b, s * NS:(s + 1) * NS].reshape(ROWS)
        idxt, ypos = _prep_core(flat)
        xb = np.ascontiguousarray(x[b].reshape(N, E))
        in_maps.append({"xb": xb, "idxt": idxt})
        yposs.append(ypos)

    nc = _get_nc()
    res = run_bass_kernel_spmd(
        nc, in_maps, core_ids=list(range(NCORES)), trace=_trace
    )
    LAST_RESULTS = res

    x_nh = np.empty((B, N, NH, NV, E), dtype=np.float32)
    for c in range(NCORES):
        b, s = divmod(c, SPB)
        dst = x_nh[b, s * NS:(s + 1) * NS].reshape(ROWS, E)
        np.take(res.results[c]["y"], yposs[c], axis=0, out=dst)

    nh_mask = adjc_mask_invalid[local_indices]            # [B, N, NH]
    mask = np.broadcast_to(nh_mask[..., None], (B, N, NH, NV)).copy()
    return x_nh, mask
